# revision 16
# baseline (speedup 1.0000x reference)
"""Trainium2 Bass kernel for nn_BilinearScorer.

Computation (reference):
    pred [n=4096, h=512], args [n, h], U [h, R=64, h], bias1 [1, R*h], bias2 [1, R]
    first = pred @ U.reshape(h, R*h) + bias1           # [n, R*h]
    out   = einsum('nrk,nk->nr', first.reshape(n,R,h), args) + bias2   # [n, R]

Sharding: tensor-parallel over the role dim R. Each of the 8 cores owns
RL = 8 roles (its U / bias slice); pred and args are replicated. Each core
produces out[:, rc:rc+8]; the host concatenates. No collectives needed.

Per-core algorithm (all matmuls bf16 with fp32 PSUM accumulation):
  for each 128-token block b:
    C_psum[tok, r]  = sum_k args[tok,k] * bias1[r,k] + bias2[r]      (PE, N=8)
    for each local role r:
      F_psum[tok, k] = sum_j pred[tok,j] * U[j,r,k]                  (PE, 4 K-tiles)
      out[tok, r]    = reduce_k(F_psum * args) + C[tok, r]           (DVE fused
                       tensor_tensor_reduce, init scalar = C column)
"""

import numpy as np
import ml_dtypes

HID = 512
ROLES = 64
N_CORES = 8
RL = ROLES // N_CORES      # local roles per core
NTOK = 8 * 512             # b*t
P = 128                    # partitions
NBLK = NTOK // P           # 32 token blocks
JT = HID // P              # 4 contraction tiles (j)
KT = HID // P              # 4 contraction tiles (k)

_BF = ml_dtypes.bfloat16
_CACHE = {}


def _make_tile_context(nc):
    """TileContext whose kernel-tail drain splits its sem waits across
    multiple single-wait Drain instructions. The walrus build in this
    container rejects a Drain carrying >(about 2) sync waits
    (CoreV3GenImpl setupSyncWait: "Too many sync wait commands")."""
    import concourse.mybir as mybir
    from concourse.tile import TileContext
    from concourse.vector_clock import ScopedClock

    class SplitDrainTileContext(TileContext):
        # Max sync-waits this walrus accepts per instruction. Excess waits
        # are hoisted onto same-engine NoOps placed immediately before.
        _WAIT_LIMIT = 1

        def _commit_instruction(self, inst, lazy_reg_writes=True):
            limit = self._WAIT_LIMIT
            si = inst.sync_info
            if limit is not None and si is not None and len(si.on_wait) > limit:
                waits = list(si.on_wait)
                excess, keep = waits[:-limit], waits[-limit:]
                for w in excess:
                    noop = mybir.InstNoOp(
                        name=self.nc.get_next_instruction_name(),
                        sync_info=mybir.SyncInfo(on_wait=[w], on_update=[]),
                        bass_nofuse=True,
                        engine=inst.engine,
                    )
                    super()._commit_instruction(noop, lazy_reg_writes=False)
                inst.sync_info = mybir.SyncInfo(
                    on_wait=keep, on_update=list(si.on_update)
                )
            return super()._commit_instruction(inst, lazy_reg_writes)

        def _drain_and_barrier(self, tick_clock, wait_clock):
            nc = self.nc
            drain_inst = nc.sync.drain()
            wait_clock.add_sem_waits(
                drain_inst.ins, ScopedClock({None: tick_clock.global_clock})
            )
            si = drain_inst.ins.sync_info
            if si is not None and len(si.on_wait) > 1:
                waits = list(si.on_wait)
                drain_inst.ins.sync_info = mybir.SyncInfo(
                    on_wait=[waits[0]], on_update=list(si.on_update)
                )
                for w in waits[1:]:
                    d2 = nc.sync.drain()
                    d2.ins.sync_info = mybir.SyncInfo(on_wait=[w], on_update=[])
            nc.all_engine_barrier()
            assert self.sems is not None
            popped = nc._tile_sem_poison_stack.pop()
            assert popped is self._sem_poison
            nc.clear_and_free_semaphores(list(self.sems.allocated().values()))
            nc.all_engine_barrier()

    return SplitDrainTileContext(nc)


def _build():
    """Build the (single-program SPMD) Bass module."""
    import concourse.bass as bass
    import concourse.mybir as mybir

    f32 = mybir.dt.float32
    bf16 = mybir.dt.bfloat16
    nc = bass.Bass()

    # DRAM I/O. Layouts are host-prepped so every DMA is partition-friendly:
    #   predt[p, jt, n]     = pred[n, jt*128+p]          (bf16)
    #   u[p, jt*RL*HID + r*HID + k] = U[jt*128+p, rc+r, k] (bf16)
    #   args[n, k]                                        (f32, natural)
    #   argst[p, kt, n]     = args[n, kt*128+p]          (bf16)
    #   b1t[p, kt*RL + r]   = bias1_2d[rc+r, kt*128+p]   (bf16)
    #   b2[0, r]            = bias2[rc+r]                (bf16)
    predt = nc.declare_dram_parameter("predt", [P, JT, NTOK], bf16, isOutput=False)
    u = nc.declare_dram_parameter("u", [P, JT * RL * HID], bf16, isOutput=False)
    args = nc.declare_dram_parameter("args", [NTOK, HID], f32, isOutput=False)
    argst = nc.declare_dram_parameter("argst", [P, KT, NTOK], bf16, isOutput=False)
    b1t = nc.declare_dram_parameter("b1t", [P, KT * RL], bf16, isOutput=False)
    b2 = nc.declare_dram_parameter("b2", [1, RL], bf16, isOutput=False)
    out = nc.declare_dram_parameter("out", [NTOK, RL], f32, isOutput=True)

    with _make_tile_context(nc) as tc:
        with (
            tc.tile_pool(name="const", bufs=1) as cpool,
            tc.tile_pool(name="pred", bufs=4) as ppool,
            tc.tile_pool(name="argsp", bufs=4) as apool,
            tc.tile_pool(name="argstp", bufs=4) as atpool,
            tc.tile_pool(name="outp", bufs=3) as opool,
            tc.tile_pool(name="misc", bufs=2) as mpool,
            tc.tile_pool(name="fps", bufs=8, space="PSUM") as fpsum,
        ):
            # Startup order matters: the tiny bias tensors and block-0 inputs
            # go at the head of the sync queue so the PE can start within a
            # few us; the 4 MiB U load is split across the sync and gpsimd
            # queues; steady-state block inputs stream on the scalar queue.
            b1t_sb = cpool.tile([P, KT * RL], bf16)
            nc.sync.dma_start(out=b1t_sb[:], in_=b1t[:])
            b2_sb = cpool.tile([1, RL], bf16)
            nc.sync.dma_start(out=b2_sb[:], in_=b2[:])
            ones_sb = cpool.tile([1, P], bf16)
            nc.vector.memset(ones_sb[:], 1.0)

            argst_sbs = {}
            pa_sbs = {}
            c_sbs = {}

            def load_argst(b, eng):
                tok = slice(b * P, (b + 1) * P)
                t = atpool.tile(
                    [P, KT, P], bf16, name="argst_sb", tag="argst_sb", bufs=6
                )
                eng.dma_start(out=t[:], in_=argst[:, :, tok])
                argst_sbs[b] = t

            def load_pa(b, eng):
                tok = slice(b * P, (b + 1) * P)
                args_sb = apool.tile([P, HID], f32, name="args_sb", tag="args_sb")
                eng.dma_start(out=args_sb[:], in_=args[tok, :])
                pred_sb = ppool.tile([P, JT, P], bf16, name="pred_sb", tag="pred_sb")
                eng.dma_start(out=pred_sb[:], in_=predt[:, :, tok])
                pa_sbs[b] = (pred_sb, args_sb)

            def c_part(b):
                argst_sb = argst_sbs.pop(b)
                # C[tok, r] = sum_k args[tok,k]*bias1[r,k] + bias2[r]
                c_ps = fpsum.tile([P, RL], f32, name="c_ps", tag="fps_tile")
                for kt in range(KT):
                    nc.tensor.matmul(
                        c_ps[:],
                        argst_sb[:, kt, :],
                        b1t_sb[:, kt * RL:(kt + 1) * RL],
                        start=(kt == 0),
                        stop=False,
                    )
                nc.tensor.matmul(c_ps[:], ones_sb[:], b2_sb[:], start=False, stop=True)
                # Evacuate C to SBUF on the idle ACT engine to free the bank.
                c_sb = mpool.tile([P, RL], f32, name="c_sb", tag="c_sb", bufs=6)
                nc.scalar.copy(out=c_sb[:], in_=c_ps[:])
                c_sbs[b] = c_sb

            # Startup: block-0 critical tensors first on the sync queue, U
            # split across the sync/gpsimd queues. argst (feeds the cheap C
            # matmuls that keep the PE warm during the U load) prefetches
            # CLOOK_C ahead on the scalar queue; the bulkier pred/args only
            # CLOOK_F ahead to limit HBM contention with the U load.
            CLOOK_C = 4
            CLOOK_F = 2
            load_argst(0, nc.sync)
            load_pa(0, nc.sync)

            seg = RL * HID
            u_sbs = []
            for jt in range(JT):
                u_t = cpool.tile(
                    [P, seg], bf16, name=f"u_sb{jt}", tag=f"u_sb{jt}"
                )
                (nc.sync if jt % 2 == 0 else nc.gpsimd).dma_start(
                    out=u_t[:], in_=u[:, jt * seg:(jt + 1) * seg]
                )
                u_sbs.append(u_t)

            for b in range(1, CLOOK_C):
                load_argst(b, nc.scalar)
            for b in range(CLOOK_C):
                c_part(b)
            for b in range(1, CLOOK_F):
                load_pa(b, nc.scalar)

            for b in range(NBLK):
                tok = slice(b * P, (b + 1) * P)
                if b + CLOOK_C < NBLK:
                    load_argst(b + CLOOK_C, nc.scalar)
                    c_part(b + CLOOK_C)
                if b + CLOOK_F < NBLK:
                    load_pa(b + CLOOK_F, nc.scalar)
                pred_sb, args_sb = pa_sbs.pop(b)
                c_sb = c_sbs.pop(b)

                acc_sb = mpool.tile([P, RL], f32, name="acc_sb", tag="acc_sb")
                out_sb = opool.tile([P, RL], f32)
                dummy = mpool.tile([P, 1], f32)
                if b < 2:
                    # Early blocks run jt-outer: the first matmuls need only
                    # u_jt0, which arrives long before the rest of U.
                    pss = [
                        fpsum.tile([P, HID], f32, name="fps_tile", tag="fps_tile")
                        for _ in range(RL)
                    ]
                    for jt in range(JT):
                        for r in range(RL):
                            nc.tensor.matmul(
                                pss[r][:],
                                pred_sb[:, jt, :],
                                u_sbs[jt][:, r * HID:(r + 1) * HID],
                                start=(jt == 0),
                                stop=(jt == JT - 1),
                            )
                    for r in range(RL):
                        nc.vector.scalar_tensor_tensor(
                            out=dummy.broadcast_to([P, HID]),
                            in0=pss[r][:],
                            scalar=1.0,
                            in1=args_sb[:],
                            op0=mybir.AluOpType.mult,
                            op1=mybir.AluOpType.mult,
                            accum_out=acc_sb[:, r:r + 1],
                        )
                else:
                    # Role-outer: each role's 4 accumulating matmuls finish
                    # back-to-back so its DVE reduce starts immediately (the
                    # per-matmul LDWEIGHTS cost is identical either way).
                    for r in range(RL):
                        ps = fpsum.tile(
                            [P, HID], f32, name="fps_tile", tag="fps_tile"
                        )
                        for jt in range(JT):
                            nc.tensor.matmul(
                                ps[:],
                                pred_sb[:, jt, :],
                                u_sbs[jt][:, r * HID:(r + 1) * HID],
                                start=(jt == 0),
                                stop=(jt == JT - 1),
                            )
                        nc.vector.scalar_tensor_tensor(
                            out=dummy.broadcast_to([P, HID]),
                            in0=ps[:],
                            scalar=1.0,
                            in1=args_sb[:],
                            op0=mybir.AluOpType.mult,
                            op1=mybir.AluOpType.mult,
                            accum_out=acc_sb[:, r:r + 1],
                        )
                nc.vector.tensor_add(out=out_sb[:], in0=acc_sb[:], in1=c_sb[:])
                nc.gpsimd.dma_start(out=out[tok, :], in_=out_sb[:])
    return nc


def _prep_in_maps(pred_input, args_input, U, bias1, bias2):
    pred = np.asarray(pred_input, np.float32).reshape(NTOK, HID)
    args = np.asarray(args_input, np.float32).reshape(NTOK, HID)
    U = np.asarray(U, np.float32)
    bias1_2d = np.asarray(bias1, np.float32).reshape(ROLES, HID)
    bias2_v = np.asarray(bias2, np.float32).reshape(ROLES)

    predt = np.ascontiguousarray(
        pred.T.reshape(JT, P, NTOK).transpose(1, 0, 2).astype(_BF)
    )
    argst = np.ascontiguousarray(
        args.T.reshape(KT, P, NTOK).transpose(1, 0, 2).astype(_BF)
    )
    args_c = np.ascontiguousarray(args)

    in_maps = []
    for c in range(N_CORES):
        rc = c * RL
        u_prep = np.ascontiguousarray(
            U[:, rc:rc + RL, :]
            .reshape(JT, P, RL, HID)
            .transpose(1, 0, 2, 3)
            .reshape(P, JT * RL * HID)
            .astype(_BF)
        )
        b1t = np.ascontiguousarray(
            bias1_2d[rc:rc + RL]
            .T.reshape(KT, P, RL)
            .transpose(1, 0, 2)
            .reshape(P, KT * RL)
            .astype(_BF)
        )
        b2c = np.ascontiguousarray(bias2_v[rc:rc + RL].reshape(1, RL).astype(_BF))
        in_maps.append(
            {
                "predt": predt,
                "u": u_prep,
                "args": args_c,
                "argst": argst,
                "b1t": b1t,
                "b2": b2c,
            }
        )
    return in_maps


def run(inputs, trace=False):
    """Run on all 8 cores; returns (full_output, BassKernelResults)."""
    from concourse.bass_utils import run_bass_kernel_spmd

    if "nc" not in _CACHE:
        _CACHE["nc"] = _build()
    in_maps = _prep_in_maps(**inputs)
    res = run_bass_kernel_spmd(
        _CACHE["nc"], in_maps, core_ids=list(range(N_CORES)), trace=trace
    )
    full = np.concatenate(
        [np.asarray(r["out"], np.float32) for r in res.results], axis=1
    )
    return full, res


def kernel(pred_input, args_input, U, bias1, bias2):
    full, _ = run(
        {
            "pred_input": pred_input,
            "args_input": args_input,
            "U": U,
            "bias1": bias1,
            "bias2": bias2,
        }
    )
    return full
